# revision 1
# baseline (speedup 1.0000x reference)
"""Single-head unscaled attention (B=8, T=2048, D=1024, NODES=1024) on 8 trn2 cores.

Sharding: data-parallel over batch — core b computes batch element b end-to-end.
Weights are replicated to every core.

Per-core pipeline (all matmuls fp16 in / fp32 PSUM accumulate):
  X^T  = PE-transpose(cast16(X))                     [d, t]
  Q^T  = Wq^T X^T,  K^T = Wk^T X^T  (lhsT=W tile)    [n, t]
  V    = X Wv       (lhsT=X^T tile)                  [t, n]
  per q-tile (128 rows):
    S    = Q^T.T K^T   -> PSUM [128, 2048]
    softmax: row-max (DVE) -> exp+row-sum fused on ACT -> P fp16
    P^T  via PE transpose (16x [128,128])
    O    = P^T.T V     -> PSUM [128, 1024];  O *= 1/rowsum;  DMA out
"""

from contextlib import ExitStack

import numpy as np

import concourse.bass as bass
import concourse.mybir as mybir
import concourse.tile as tile
from concourse import bacc
from concourse.bass import ts
from concourse.masks import make_identity

P = 128
T = 2048
D = 1024
NO = 1024
B = 8
TT = T // P   # 16 tiles of 128 along t
DT = D // P   # 8 tiles along d
NT = NO // P  # 8 tiles along nodes

F16 = mybir.dt.float16
F32 = mybir.dt.float32
AX = mybir.AxisListType
EXP = mybir.ActivationFunctionType.Exp


def _attention_body(tc, out, x, wq, wk, wv):
    nc = tc.nc
    x3 = x.rearrange("(t p) d -> t p d", p=P)
    o3 = out.rearrange("(t p) n -> t p n", p=P)

    with ExitStack() as ctx:
        const = ctx.enter_context(tc.tile_pool(name="const", bufs=1))
        persist = ctx.enter_context(tc.tile_pool(name="persist", bufs=1))
        # shared 1-bank psum slots: projection accumulators + transposes
        ppsum = ctx.enter_context(tc.tile_pool(name="ppsum", bufs=2, space="PSUM"))

        ident = const.tile([P, P], F16, tag="ident")
        make_identity(nc, ident)

        xt = persist.tile([P, DT, T], F16, tag="xt")    # X^T [d_in, d_out, t]
        qt = persist.tile([P, NT, T], F16, tag="qt")    # Q^T [n_in, n_out, t]
        kt = persist.tile([P, NT, T], F16, tag="kt")    # K^T
        v = persist.tile([P, TT, NO], F16, tag="v")     # V   [t_in, t_out, n]

        with tc.tile_pool(name="stage", bufs=2) as stage, tc.tile_pool(
            name="wpool", bufs=2
        ) as wpool:
            # ---- X^T: load, cast fp16, PE-transpose 128x128 blocks
            for t_ in range(TT):
                xs = stage.tile([P, D], F32, tag="xs")
                nc.sync.dma_start(xs, x3[t_])
                xh = stage.tile([P, D], F16, tag="xh")
                nc.scalar.copy(xh, xs)
                for do in range(DT):
                    tp = ppsum.tile([P, P], F16, tag="pp")
                    nc.tensor.transpose(tp, xh[:, ts(do, P)], ident)
                    nc.vector.tensor_copy(xt[:, do, ts(t_, P)], tp)

            def load_w(wap):
                w16 = wpool.tile([P, DT, NO], F16, tag="w16")
                w3 = wap.rearrange("(do p) n -> do p n", p=P)
                for do in range(DT):
                    wsta = stage.tile([P, NO], F32, tag="ws")
                    nc.sync.dma_start(wsta, w3[do])
                    nc.scalar.copy(w16[:, do, :], wsta)
                return w16

            # ---- Q^T, K^T: lhsT = W[d, n-tile], rhs = X^T[d, q-block]
            for w_ap, dst in ((wq, qt), (wk, kt)):
                w16 = load_w(w_ap)
                for no in range(NT):
                    for qb in range(4):
                        ps = ppsum.tile([P, 512], F32, tag="pp")
                        for do in range(DT):
                            nc.tensor.matmul(
                                ps,
                                w16[:, do, ts(no, P)],
                                xt[:, do, ts(qb, 512)],
                                start=(do == 0),
                                stop=(do == DT - 1),
                            )
                        nc.vector.tensor_copy(dst[:, no, ts(qb, 512)], ps)

            # ---- V: lhsT = X^T[d, t-tile], rhs = Wv[d, n-block]
            wv16 = load_w(wv)
            for t_ in range(TT):
                for nb in range(2):
                    ps = ppsum.tile([P, 512], F32, tag="pp")
                    for do in range(DT):
                        nc.tensor.matmul(
                            ps,
                            xt[:, do, ts(t_, P)],
                            wv16[:, do, ts(nb, 512)],
                            start=(do == 0),
                            stop=(do == DT - 1),
                        )
                    nc.vector.tensor_copy(v[:, t_, ts(nb, 512)], ps)

        # ---- attention per q-tile
        with tc.tile_pool(name="spsum", bufs=1, space="PSUM") as spsum, tc.tile_pool(
            name="opsum", bufs=1, space="PSUM"
        ) as opsum, tc.tile_pool(name="soft", bufs=2) as soft, tc.tile_pool(
            name="ptp", bufs=2
        ) as ptpool, tc.tile_pool(name="outp", bufs=2) as outp:
            for q_ in range(TT):
                s = spsum.tile([P, 4, 512], F32, tag="s")
                bmax = soft.tile([P, 4], F32, tag="bmax")
                for kb in range(4):
                    for no in range(NT):
                        nc.tensor.matmul(
                            s[:, kb],
                            qt[:, no, ts(q_, P)],
                            kt[:, no, ts(kb, 512)],
                            start=(no == 0),
                            stop=(no == NT - 1),
                        )
                    # block max as soon as this 512-block of S is done
                    nc.vector.tensor_reduce(
                        bmax[:, kb : kb + 1], s[:, kb], axis=AX.X, op=mybir.AluOpType.max
                    )
                rmax = soft.tile([P, 1], F32, tag="rmax")
                nc.vector.tensor_reduce(rmax, bmax, axis=AX.X, op=mybir.AluOpType.max)
                negmax = soft.tile([P, 1], F32, tag="negmax")
                nc.vector.tensor_scalar_mul(negmax, rmax, -1.0)

                p16 = soft.tile([P, T], F16, tag="p16")
                bsum = soft.tile([P, 4], F32, tag="bsum")
                for kb in range(4):
                    nc.scalar.activation(
                        p16[:, ts(kb, 512)],
                        s[:, kb],
                        EXP,
                        bias=negmax,
                        scale=1.0,
                        accum_out=bsum[:, kb : kb + 1],
                    )
                rsum = soft.tile([P, 1], F32, tag="rsum")
                nc.vector.tensor_reduce(rsum, bsum, axis=AX.X, op=mybir.AluOpType.add)
                inv = soft.tile([P, 1], F32, tag="inv")
                nc.vector.reciprocal(inv, rsum)

                ptt = ptpool.tile([P, TT, P], F16, tag="ptt")
                for k_ in range(TT):
                    tp = ppsum.tile([P, P], F16, tag="pp")
                    nc.tensor.transpose(tp, p16[:, ts(k_, P)], ident)
                    nc.vector.tensor_copy(ptt[:, k_, :], tp)

                o = opsum.tile([P, 2, 512], F32, tag="o")
                for nb in range(2):
                    for k_ in range(TT):
                        nc.tensor.matmul(
                            o[:, nb],
                            ptt[:, k_, :],
                            v[:, k_, ts(nb, 512)],
                            start=(k_ == 0),
                            stop=(k_ == TT - 1),
                        )
                ob = outp.tile([P, NO], F32, tag="ob")
                for nb in range(2):
                    nc.vector.tensor_scalar_mul(ob[:, ts(nb, 512)], o[:, nb], inv)
                nc.sync.dma_start(o3[q_], ob)


_CACHED_NC = None


def _build():
    global _CACHED_NC
    if _CACHED_NC is not None:
        return _CACHED_NC
    nc = bacc.Bacc("TRN2", target_bir_lowering=False, debug=False, num_devices=1)
    x = nc.dram_tensor("x", (T, D), F32, kind="ExternalInput").ap()
    wq = nc.dram_tensor("wq", (D, NO), F32, kind="ExternalInput").ap()
    wk = nc.dram_tensor("wk", (D, NO), F32, kind="ExternalInput").ap()
    wv = nc.dram_tensor("wv", (D, NO), F32, kind="ExternalInput").ap()
    out = nc.dram_tensor("out", (T, NO), F32, kind="ExternalOutput").ap()
    with tile.TileContext(nc) as tc:
        _attention_body(tc, out, x, wq, wk, wv)
    nc.compile()
    _CACHED_NC = nc
    return nc


def kernel(inputs, Wq, Wk, Wv, trace=False):
    from concourse.bass_utils import run_bass_kernel_spmd

    nc = _build()
    inputs = np.ascontiguousarray(inputs, dtype=np.float32)
    Wq = np.ascontiguousarray(Wq, dtype=np.float32)
    Wk = np.ascontiguousarray(Wk, dtype=np.float32)
    Wv = np.ascontiguousarray(Wv, dtype=np.float32)
    in_maps = [
        {"x": inputs[b], "wq": Wq, "wk": Wk, "wv": Wv} for b in range(B)
    ]
    res = run_bass_kernel_spmd(nc, in_maps, core_ids=list(range(B)), trace=trace)
    out = np.stack([r["out"] for r in res.results], axis=0)
    if trace:
        kernel.last_results = res
    return out

